# revision 1
# baseline (speedup 1.0000x reference)
"""GAU-alpha (gated attention unit) Trainium2 kernel, fp8 DoubleRow edition.

Data-parallel over batch: 64 batches -> 8 NeuronCores x 8 batches.
Per-batch pipeline on-chip:
  (host) ScaleNorm + transpose + fp8 hi/lo split of xn ->
  fused uv projection + SiLU -> gamma/beta + RoPE (all-bf16 DVE) ->
  relu^2 relative-position attention -> gating -> output projection + residual.

All large matmuls run as fp8e4m3 DoubleRow instructions (2 (weight, ifmap)
slot-pairs per instruction).  Each operand X is split into hi+lo fp8 tiles
(X ~= X8 + Xr8, bf16-level accuracy); each K=128 block contributes 3 pairs:
(W8,A8), (Wr8,A8), (W8,Ar8).  The qk matmul stays bf16 (small).  Scales:
uvw*16 (silu scale 1/16), relu scale 1/32 -> ker_dev = ker_ref/8, o_w*8,
so no epilogue descale is needed.
"""
import numpy as np
import ml_dtypes

import concourse.bass as bass
import concourse.tile as tile
from concourse import mybir
from concourse.bass_utils import run_bass_kernel_spmd

F32 = mybir.dt.float32
F32R = mybir.dt.float32r
BF16 = mybir.dt.bfloat16
F8 = mybir.dt.float8e4
E4NP = ml_dtypes.float8_e4m3
BF16NP = ml_dtypes.bfloat16

B, S, H = 64, 512, 512
E = 1024          # expansion dim
SD = 128          # attention head width s
UV = 2 * E + SD   # 2176
N_CORES = 8
BPC = B // N_CORES
P = 128
EPS = 1e-5
DR = mybir.MatmulPerfMode.DoubleRow


def _split_waits(nc, max_waits=1):
    """This walrus build rejects >1 sync-wait on CTRL-encoded instructions
    (Drain/NoOp); Tile's exit drain always violates that.  Split any
    instruction carrying more than `max_waits` waits into a chain of
    single-wait NoOps on the same engine."""
    ctr = 0
    for f in nc.m.functions:
        for bb in f.blocks:
            new_insts = []
            for ins in bb.instructions:
                si = ins.sync_info
                if si is not None and si.on_wait and len(si.on_wait) > max_waits:
                    waits = list(si.on_wait)
                    head, tail = waits[:-max_waits], waits[-max_waits:]
                    for w in head:
                        ctr += 1
                        nop = mybir.InstNoOp(
                            name=f"I-waitsplit-{ctr}",
                            ins=[], outs=[],
                            sync_info=mybir.SyncInfo(on_wait=[w], on_update=[]),
                        )
                        nop.engine = ins.engine
                        new_insts.append(nop)
                    si.on_wait = tail
                new_insts.append(ins)
            bb.instructions = new_insts
    return ctr


def _build_program(sim_compat=False, split=True):
    nc = bass.Bass()
    AF = mybir.ActivationFunctionType
    ALU = mybir.AluOpType

    x_d = nc.dram_tensor("x8", [BPC, S, H], F32R, kind="ExternalInput")
    xn8_d = nc.dram_tensor("xn8", [BPC, 2, P, 2 * S], F8, kind="ExternalInput")
    xnr8_d = nc.dram_tensor("xnr8", [BPC, 2, P, 2 * S], F8, kind="ExternalInput")
    uvw8_d = nc.dram_tensor("uvw8", [2, P, 2 * UV], F8, kind="ExternalInput")
    uvwr8_d = nc.dram_tensor("uvwr8", [2, P, 2 * UV], F8, kind="ExternalInput")
    ow8_d = nc.dram_tensor("ow8", [4, P, 2 * H], F8, kind="ExternalInput")
    owr8_d = nc.dram_tensor("owr8", [4, P, 2 * H], F8, kind="ExternalInput")
    biasT_d = nc.dram_tensor("biasT", [S, S], F32R, kind="ExternalInput")
    idr_d = nc.dram_tensor("identr", [P, P], F32R, kind="ExternalInput")
    cc_d = nc.dram_tensor("cc", [P, S], BF16, kind="ExternalInput")
    ss_d = nc.dram_tensor("ss", [P, S], BF16, kind="ExternalInput")
    bq_d = nc.dram_tensor("bq", [P, S], BF16, kind="ExternalInput")
    bk_d = nc.dram_tensor("bk", [P, S], BF16, kind="ExternalInput")
    gsc_d = nc.dram_tensor("gsc", [P, 4], F32, kind="ExternalInput")
    y_d = nc.dram_tensor("y8", [BPC, S, H], F32, kind="ExternalOutput")

    with tile.TileContext(nc) as tc:
        with (
            tc.tile_pool(name="const", bufs=1) as cst,
            tc.tile_pool(name="work", bufs=2) as wk,
            tc.tile_pool(name="ps", bufs=4, space="PSUM") as pp,
        ):
            # ---- constants into SBUF (small tables first, spread queues) ----
            biasT = [cst.tile([P, S], F32R, tag=f"biasT{j}", name=f"biasT{j}") for j in range(4)]
            identr = cst.tile([P, P], F32R, tag="identr", name="identr")
            cc = cst.tile([P, S], BF16, tag="cc", name="cc")
            ssn = cst.tile([P, S], BF16, tag="ssn", name="ssn")
            bq = cst.tile([P, S], BF16, tag="bq", name="bq")
            bk = cst.tile([P, S], BF16, tag="bk", name="bk")
            gsc = cst.tile([P, 4], F32, tag="gsc", name="gsc")
            nc.scalar.dma_start(out=cc[:], in_=cc_d[:])
            nc.scalar.dma_start(out=ssn[:], in_=ss_d[:])
            nc.scalar.dma_start(out=bq[:], in_=bq_d[:])
            nc.scalar.dma_start(out=bk[:], in_=bk_d[:])
            nc.scalar.dma_start(out=gsc[:], in_=gsc_d[:])
            uvw8 = [cst.tile([P, 2, UV], F8, tag=f"uvw8{k}", name=f"uvw8{k}") for k in range(2)]
            uvwr8 = [cst.tile([P, 2, UV], F8, tag=f"uvwr8{k}", name=f"uvwr8{k}") for k in range(2)]
            nc.gpsimd.dma_start(out=uvw8[0][:], in_=uvw8_d[0])
            nc.scalar.dma_start(out=uvw8[1][:], in_=uvw8_d[1])
            nc.sync.dma_start(out=uvwr8[0][:], in_=uvwr8_d[0])
            nc.scalar.dma_start(out=uvwr8[1][:], in_=uvwr8_d[1])
            for j in range(4):
                nc.gpsimd.dma_start(out=biasT[j][:], in_=biasT_d[j * P:(j + 1) * P, :])
            nc.scalar.dma_start(out=identr[:], in_=idr_d[:])
            ow8 = [cst.tile([P, 2, H], F8, tag=f"ow8{e}", name=f"ow8{e}") for e in range(4)]
            owr8 = [cst.tile([P, 2, H], F8, tag=f"owr8{e}", name=f"owr8{e}") for e in range(4)]
            for e in range(4):
                nc.scalar.dma_start(out=ow8[e][:], in_=ow8_d[e])
                nc.gpsimd.dma_start(out=owr8[e][:], in_=owr8_d[e])

            pro = {}

            def prologue(b):
                xn8 = [wk.tile([P, 2, S], F8, tag=f"xn8{k}", name=f"xn8{k}") for k in range(2)]
                xnr8 = [wk.tile([P, 2, S], F8, tag=f"xnr8{k}", name=f"xnr8{k}") for k in range(2)]
                for k in range(2):
                    nc.sync.dma_start(out=xn8[k][:], in_=xn8_d[b, k])
                    nc.sync.dma_start(out=xnr8[k][:], in_=xnr8_d[b, k])
                xpair = [wk.tile([P, 2, H], F32R, tag=f"xp{sp}", name=f"xp{sp}", bufs=4)
                         for sp in range(2)]
                for sp in range(2):
                    for hf in range(2):
                        st = 2 * sp + hf
                        nc.sync.dma_start(out=xpair[sp][:, hf, :],
                                          in_=x_d[b, st * P:(st + 1) * P, :])
                pro[b] = (xn8, xnr8, xpair)

            st_a = {}
            st_k = {}
            st_g = {}

            def phase_a(b):
                """projection phase: base/rope first, then u, then v"""
                xn8, xnr8, xpair = pro.pop(b)

                def uv_seq():
                    return ([(uvw8[k], xn8[k]) for k in range(2)]
                            + [(uvwr8[k], xn8[k]) for k in range(2)]
                            + [(uvw8[k], xnr8[k]) for k in range(2)])

                # base projection (cols 2E:2E+P) + rotation + rope tables
                psb = pp.tile([P, 2, 512], F32, tag="ps", name="ps")
                for i, (w, a) in enumerate(uv_seq()):
                    nc.tensor.matmul(
                        psb[:, 0, :], w[:, :, 2 * E:2 * E + P], a[:],
                        start=(i == 0), stop=(i == 5), perf_mode=DR)
                baseT = wk.tile([P, S], BF16, tag="baseT", name="baseT")
                nc.scalar.activation(out=baseT[:], in_=psb[:, 0, :],
                                     func=AF.Silu, scale=1 / 16.)
                base_sw = wk.tile([P, S], BF16, tag="base_sw", name="base_sw")
                nc.sync.dma_start(out=base_sw[0:64, :], in_=baseT[64:128, :])
                nc.sync.dma_start(out=base_sw[64:128, :], in_=baseT[0:64, :])
                qT = wk.tile([P, S], BF16, tag="qT", name="qT")
                kTt = wk.tile([P, S], BF16, tag="kT", name="kT")
                for hd, (Bt, dst) in enumerate(((bq, qT), (bk, kTt))):
                    m1 = wk.tile([P, S], BF16, tag="m1", name="m1")
                    m2 = wk.tile([P, S], BF16, tag="m2", name="m2")
                    nc.vector.scalar_tensor_tensor(
                        out=m1[:], in0=baseT[:], scalar=gsc[:, 2 * hd:2 * hd + 1],
                        in1=cc[:], op0=ALU.mult, op1=ALU.mult)
                    nc.vector.scalar_tensor_tensor(
                        out=m2[:], in0=base_sw[:], scalar=gsc[:, 2 * hd + 1:2 * hd + 2],
                        in1=ssn[:], op0=ALU.mult, op1=ALU.mult)
                    nc.vector.tensor_tensor(out=m1[:], in0=m1[:], in1=m2[:], op=ALU.add)
                    nc.vector.tensor_tensor(out=dst[:], in0=m1[:], in1=Bt[:], op=ALU.add)

                uT = [wk.tile([P, 2, S], BF16, tag=f"uT{fp}", name=f"uT{fp}") for fp in range(4)]
                for fp in range(4):
                    ps = pp.tile([P, 2, 512], F32, tag="ps", name="ps")
                    for hf in range(2):
                        f = 2 * fp + hf
                        for i, (w, a) in enumerate(uv_seq()):
                            nc.tensor.matmul(
                                ps[:, hf, :], w[:, :, f * P:(f + 1) * P], a[:],
                                start=(i == 0), stop=(i == 5), perf_mode=DR)
                    nc.scalar.activation(out=uT[fp][:], in_=ps[:],
                                         func=AF.Silu, scale=1 / 16.)

                # v projection, natural [s_j, e]; fp8 hi/lo
                v8 = [wk.tile([P, 2, E], F8, tag=f"v8{j2}", name=f"v8{j2}") for j2 in range(2)]
                vr8 = [wk.tile([P, 2, E], F8, tag=f"vr8{j2}", name=f"vr8{j2}") for j2 in range(2)]
                for j in range(4):
                    ps = pp.tile([P, 2, 512], F32, tag="ps", name="ps")
                    for ec in range(2):
                        seq = ([(xn8[k], uvw8[k]) for k in range(2)]
                               + [(xn8[k], uvwr8[k]) for k in range(2)]
                               + [(xnr8[k], uvw8[k]) for k in range(2)])
                        for i, (a, w) in enumerate(seq):
                            nc.tensor.matmul(
                                ps[:, ec, :], a[:, :, j * P:(j + 1) * P],
                                w[:, :, E + ec * 512:E + (ec + 1) * 512],
                                start=(i == 0), stop=(i == 5), perf_mode=DR)
                    vf = wk.tile([P, E], BF16, tag="vf", name="vf")
                    nc.scalar.activation(out=vf[:], in_=ps[:],
                                         func=AF.Silu, scale=1 / 16.)
                    nc.vector.tensor_copy(out=v8[j // 2][:, j % 2, :], in_=vf[:])
                    nc.gpsimd.tensor_tensor(out=vr8[j // 2][:, j % 2, :], in0=vf[:],
                                            in1=v8[j // 2][:, j % 2, :], op=ALU.subtract)

                if b + 1 < BPC:
                    prologue(b + 1)
                st_a[b] = (uT, v8, vr8, xpair, qT, kTt)

            def phase_qk(b):
                """scoresT + bias -> relu (1/32) -> square -> fp8 pair"""
                uT, v8, vr8, xpair, qT, kTt = st_a.pop(b)
                ker8 = [wk.tile([P, 2, S], F8, tag=f"ker8{j2}", name=f"ker8{j2}") for j2 in range(2)]
                kerr8 = [wk.tile([P, 2, S], F8, tag=f"kerr8{j2}", name=f"kerr8{j2}") for j2 in range(2)]
                for j2 in range(2):
                    ps = pp.tile([P, 2, 512], F32, tag="ps", name="ps")
                    for jh in range(2):
                        j = 2 * j2 + jh
                        nc.tensor.matmul(ps[:, jh, :], identr[:], biasT[j][:],
                                         start=True, stop=False)
                        nc.tensor.matmul(ps[:, jh, :], kTt[:, j * P:(j + 1) * P], qT[:],
                                         start=False, stop=True)
                    rl = wk.tile([P, 2, S], BF16, tag="rl", name="rl")
                    nc.scalar.activation(out=rl[:], in_=ps[:],
                                         func=AF.Relu, scale=1 / 32.)
                    kf = wk.tile([P, 2, S], BF16, tag="kf", name="kf")
                    nc.scalar.activation(out=kf[:], in_=rl[:], func=AF.Square)
                    nc.vector.tensor_copy(out=ker8[j2][:], in_=kf[:])
                    nc.gpsimd.tensor_tensor(out=kerr8[j2][:], in0=kf[:],
                                            in1=ker8[j2][:], op=ALU.subtract)
                st_k[b] = (uT, v8, vr8, ker8, kerr8, xpair)

            def phase_attn(b):
                """attention + gating for batch b (pairs produced in phase_a(b))"""
                uT, v8, vr8, ker8, kerr8, xpair = st_k.pop(b)
                g8 = [wk.tile([P, 2, S], F8, tag=f"g8{ep}", name=f"g8{ep}") for ep in range(4)]
                gr8 = [wk.tile([P, 2, S], F8, tag=f"gr8{ep}", name=f"gr8{ep}") for ep in range(4)]
                for ep in range(4):
                    ps = pp.tile([P, 2, 512], F32, tag="ps", name="ps")
                    for hf in range(2):
                        e = 2 * ep + hf
                        seq = ([(v8[j2], ker8[j2]) for j2 in range(2)]
                               + [(vr8[j2], ker8[j2]) for j2 in range(2)]
                               + [(v8[j2], kerr8[j2]) for j2 in range(2)])
                        for i, (v_, k_) in enumerate(seq):
                            nc.tensor.matmul(
                                ps[:, hf, :], v_[:, :, e * P:(e + 1) * P], k_[:],
                                start=(i == 0), stop=(i == len(seq) - 1),
                                perf_mode=DR)
                    gf = wk.tile([P, 2, S], BF16, tag="gf", name="gf")
                    nc.vector.tensor_tensor(out=gf[:], in0=ps[:],
                                            in1=uT[ep][:], op=ALU.mult)
                    nc.scalar.copy(out=g8[ep][:], in_=gf[:])
                    nc.vector.tensor_tensor(out=gr8[ep][:], in0=gf[:],
                                            in1=g8[ep][:], op=ALU.subtract)
                st_g[b] = (g8, gr8, xpair)

            def phase_out(b):
                """output projection + residual for batch b"""
                g8, gr8, xpair = st_g.pop(b)
                for sp in range(2):
                    ps = pp.tile([P, 2, 512], F32, tag="ps", name="ps")
                    for hf in range(2):
                        st = 2 * sp + hf
                        seq = ([(g8[e2], ow8[e2]) for e2 in range(4)]
                               + [(gr8[e2], ow8[e2]) for e2 in range(4)]
                               + [(g8[e2], owr8[e2]) for e2 in range(4)])
                        for i, (g_, o_) in enumerate(seq):
                            nc.tensor.matmul(
                                ps[:, hf, :], g_[:, :, st * P:(st + 1) * P], o_[:],
                                start=(i == 0), stop=(i == len(seq) - 1),
                                perf_mode=DR)
                    ysb = wk.tile([P, 2, H], F32, tag=f"ysb{sp}", name=f"ysb{sp}")
                    nc.vector.tensor_tensor(out=ysb[:], in0=ps[:],
                                            in1=xpair[sp][:].bitcast(F32),
                                            op=ALU.add)
                    for hf in range(2):
                        st = 2 * sp + hf
                        q = nc.sync if hf == 0 else nc.scalar
                        q.dma_start(out=y_d[b, st * P:(st + 1) * P, :],
                                    in_=ysb[:, hf, :])

            prologue(0)
            for b in range(BPC + 2):
                if b < BPC:
                    phase_a(b)
                if b < BPC:
                    phase_qk(b)
                if 1 <= b <= BPC:
                    phase_attn(b - 1)
                if b >= 2:
                    phase_out(b - 2)

    if split:
        _split_waits(nc)
    return nc


_CACHE = {}


def _get_program(sim_compat=False):
    key = sim_compat
    if key not in _CACHE:
        _CACHE[key] = _build_program(sim_compat)
    return _CACHE[key]


def _pair8(a):
    hi = np.asarray(a, np.float32).astype(E4NP)
    lo = (a - hi.astype(np.float32)).astype(E4NP)
    return hi, lo


def _rope_tables():
    """sin/cos must replicate the reference's jax-CPU f32 sin/cos (XLA's
    fast sin diverges from libm for large args); fall back to numpy."""
    half = SD // 2
    pos = np.arange(S, dtype=np.float32)
    inv_freq = (10000.0 ** (np.arange(half, dtype=np.float32) / half)).astype(np.float32)
    sinus = (pos[:, None] * inv_freq[None, :]).astype(np.float32)  # [S, 64]
    try:
        import jax
        cpu = jax.local_devices(backend="cpu")[0]
        with jax.default_device(cpu):
            import jax.numpy as jnp
            sv = jax.device_put(sinus, cpu)
            sin_t = np.asarray(jnp.sin(sv)).T
            cos_t = np.asarray(jnp.cos(sv)).T
    except Exception:
        sin_t = np.sin(sinus).T
        cos_t = np.cos(sinus).T
    cc = np.concatenate([cos_t, cos_t], axis=0)      # [128, S]
    ss2 = np.concatenate([-sin_t, sin_t], axis=0)    # [-sin; +sin]
    return cc.astype(np.float32), ss2.astype(np.float32)


def _host_prep(ln_g, uv_w, uv_b, gamma, beta, w_bias, o_w, o_b):
    assert np.all(uv_b == 0.0), "kernel assumes uv_b == 0"
    assert np.all(o_b == 0.0), "kernel assumes o_b == 0"
    uvwTs = (uv_w.astype(np.float64) * float(ln_g[0]) * 16.0).T.astype(np.float32)  # [H, UV]
    uvr = uvwTs.reshape(2, 2, P, UV).transpose(0, 2, 1, 3)  # [kb2, p, slot, c]
    uvw8, uvwr8 = _pair8(uvr)
    ows = (o_w.astype(np.float64) * 8.0).T.astype(np.float32)  # [E, H]
    owr = ows.reshape(4, 2, P, H).transpose(0, 2, 1, 3)
    ow8, owr8 = _pair8(owr)
    jj = np.arange(S)[:, None]
    ii = np.arange(S)[None, :]
    biasT = w_bias[jj - ii + S - 1].astype(np.float32)
    cc, ss2 = _rope_tables()

    def _sw(v):
        return np.concatenate([v[SD // 2:], v[:SD // 2]])
    bq = (beta[0][:, None] * cc + _sw(beta[0])[:, None] * ss2).astype(BF16NP)
    bk = (beta[1][:, None] * cc + _sw(beta[1])[:, None] * ss2).astype(BF16NP)
    gsc = np.stack([gamma[0], _sw(gamma[0]), gamma[1], _sw(gamma[1])],
                   axis=1).astype(np.float32)
    return {
        "uvw8": np.ascontiguousarray(uvw8.reshape(2, P, 2 * UV)),
        "uvwr8": np.ascontiguousarray(uvwr8.reshape(2, P, 2 * UV)),
        "ow8": np.ascontiguousarray(ow8.reshape(4, P, 2 * H)),
        "owr8": np.ascontiguousarray(owr8.reshape(4, P, 2 * H)),
        "biasT": biasT,
        "identr": np.eye(P, dtype=np.float32),
        "cc": cc.astype(BF16NP), "ss": ss2.astype(BF16NP),
        "bq": bq, "bk": bk, "gsc": gsc,
    }


def kernel(x, ln_g, uv_w, uv_b, gamma, beta, w_bias, o_w, o_b):
    x = np.asarray(x, dtype=np.float32)
    consts = _host_prep(np.asarray(ln_g), np.asarray(uv_w), np.asarray(uv_b),
                        np.asarray(gamma), np.asarray(beta),
                        np.asarray(w_bias), np.asarray(o_w), np.asarray(o_b))
    nc = _get_program(sim_compat=False)
    nrm = np.sqrt(np.einsum("bsh,bsh->bs", x, x, dtype=np.float32,
                            optimize=True)) * np.float32(H ** -0.5)
    inv = (1.0 / np.maximum(nrm, np.float32(EPS))).astype(np.float32)
    xn = x * inv[:, :, None]
    xnT = np.ascontiguousarray(xn.transpose(0, 2, 1))  # [B, H, S] f32
    xnr = xnT.reshape(B, 2, 2, P, S).transpose(0, 1, 3, 2, 4)  # [B, kb2, p, slot, s]
    xn8, xnr8 = _pair8(xnr)
    xn8 = np.ascontiguousarray(xn8.reshape(B, 2, P, 2 * S))
    xnr8 = np.ascontiguousarray(xnr8.reshape(B, 2, P, 2 * S))
    in_maps = []
    for c in range(N_CORES):
        m = dict(consts)
        m["x8"] = np.ascontiguousarray(x[c * BPC:(c + 1) * BPC])
        m["xn8"] = xn8[c * BPC:(c + 1) * BPC]
        m["xnr8"] = xnr8[c * BPC:(c + 1) * BPC]
        in_maps.append(m)
    res = run_bass_kernel_spmd(nc, in_maps, core_ids=list(range(N_CORES)))
    out = np.concatenate([r["y8"] for r in res.results], axis=0)
    return out.astype(np.float32)



# revision 44
# speedup vs baseline: 1.0082x; 1.0082x over previous
"""GAU-alpha (gated attention unit) Trainium2 kernel, fp8 DoubleRow edition.

Data-parallel over batch: 64 batches -> 8 NeuronCores x 8 batches.
Per-batch pipeline on-chip:
  (host) ScaleNorm + transpose + fp8 hi/lo split of xn ->
  fused uv projection + SiLU -> gamma-folded RoPE (DVE) ->
  relu^2 relative-position attention -> gating -> output projection + residual.

All large matmuls run as fp8e4m3 DoubleRow instructions (2 (weight, ifmap)
slot-pairs per instruction).  Each operand X is split into hi+lo fp8 tiles
(X ~= X8 + Xr8, bf16-level accuracy); each K=128 block contributes 3 pairs:
(W8,A8), (Wr8,A8), (W8,Ar8).  The qk matmul stays bf16 (small).  Scales:
uvw*16 (silu scale 1/16), relu scale 1/32 -> ker_dev = ker_ref/8, o_w*8,
so no epilogue descale is needed.

Engine layout per batch (cost-model):  PE ~22.8us (uv 10.9 + qk 1.7 +
attn 5.1 + out 5.1), DVE ~19us (rope TTs, relu^2 square, fp8 copies +
v/ker residual subs, gating mult), Act ~15us (SiLUs, relu, g8 copies),
Pool ~13us (g residual subs, residual adds).  Phase order per iteration
is a(b), attn(b-1), out(b-2), qk(b): qk last gives its Act/DVE epilogue
a full iteration of slack before attn(b) consumes ker8(b), and gf(b-1)
lands early enough on DVE to recycle PSUM banks for out(b-2).
DMAs are merged via host-side layouts (one descriptor per logical load)
because each HWDGE trigger costs ~0.6-1.3us of serial queue time.
"""
import numpy as np
import ml_dtypes

import concourse.bass as bass
import concourse.tile as tile
from concourse import mybir
from concourse.bass_utils import run_bass_kernel_spmd

F32 = mybir.dt.float32
F32R = mybir.dt.float32r
BF16 = mybir.dt.bfloat16
F8 = mybir.dt.float8e4
E4NP = ml_dtypes.float8_e4m3
BF16NP = ml_dtypes.bfloat16

B, S, H = 64, 512, 512
E = 1024          # expansion dim
SD = 128          # attention head width s
UV = 2 * E + SD   # 2176
N_CORES = 8
BPC = B // N_CORES
P = 128
EPS = 1e-5
DR = mybir.MatmulPerfMode.DoubleRow


def _split_waits(nc, max_waits=1):
    """This walrus build rejects >1 sync-wait on CTRL-encoded instructions
    (Drain/NoOp); Tile's exit drain always violates that.  Split any
    instruction carrying more than `max_waits` waits into a chain of
    single-wait NoOps on the same engine."""
    ctr = 0
    for f in nc.m.functions:
        for bb in f.blocks:
            new_insts = []
            for ins in bb.instructions:
                si = ins.sync_info
                if si is not None and si.on_wait and len(si.on_wait) > max_waits:
                    waits = list(si.on_wait)
                    head, tail = waits[:-max_waits], waits[-max_waits:]
                    for w in head:
                        ctr += 1
                        nop = mybir.InstNoOp(
                            name=f"I-waitsplit-{ctr}",
                            ins=[], outs=[],
                            sync_info=mybir.SyncInfo(on_wait=[w], on_update=[]),
                        )
                        nop.engine = ins.engine
                        new_insts.append(nop)
                    si.on_wait = tail
                new_insts.append(ins)
            bb.instructions = new_insts
    return ctr


def _build_program(sim_compat=False, split=True):
    nc = bass.Bass()
    AF = mybir.ActivationFunctionType
    ALU = mybir.AluOpType

    x2_d = nc.dram_tensor("x2", [BPC, 2, P, 2, H], F32R, kind="ExternalInput")
    xn8_d = nc.dram_tensor("xn8", [BPC, P, 4, S], F8, kind="ExternalInput")
    xnr8_d = nc.dram_tensor("xnr8", [BPC, P, 4, S], F8, kind="ExternalInput")
    uvb8_d = nc.dram_tensor("uvb8", [P, 4, SD], F8, kind="ExternalInput")
    uvbr8_d = nc.dram_tensor("uvbr8", [P, 4, SD], F8, kind="ExternalInput")
    uvu8_d = nc.dram_tensor("uvu8", [P, 4, E], F8, kind="ExternalInput")
    uvur8_d = nc.dram_tensor("uvur8", [P, 4, E], F8, kind="ExternalInput")
    uvv8_d = nc.dram_tensor("uvv8", [P, 4, E], F8, kind="ExternalInput")
    uvvr8_d = nc.dram_tensor("uvvr8", [P, 4, E], F8, kind="ExternalInput")
    ow8_d = nc.dram_tensor("ow8", [P, 8, H], F8, kind="ExternalInput")
    owr8_d = nc.dram_tensor("owr8", [P, 8, H], F8, kind="ExternalInput")
    biasT_d = nc.dram_tensor("biasT", [P, 4, S], BF16, kind="ExternalInput")
    rope_d = nc.dram_tensor("ropeT", [P, 6, S], BF16, kind="ExternalInput")
    idr_d = nc.dram_tensor("identr", [P, P], BF16, kind="ExternalInput")
    y_d = nc.dram_tensor("y8", [BPC, 2, P, 2, H], F32, kind="ExternalOutput")

    with tile.TileContext(nc) as tc:
        with (
            tc.tile_pool(name="const", bufs=1) as cst,
            tc.tile_pool(name="work", bufs=2) as wk,
            tc.tile_pool(name="ps", bufs=3, space="PSUM") as pp,
            tc.tile_pool(name="psq", bufs=2, space="PSUM") as pq,
        ):
            # ---- PE warm-up: keep the tensor engine streaming while the
            # first DMAs land so the p-state ramp is done before real work.
            # wtile is memset on-chip (no DMA) so warm-up starts ~immediately.
            wtile = cst.tile([P, 256], BF16, tag="wtile", name="wtile")
            nc.gpsimd.memset(wtile[:], 0)
            pw = pp.tile([P, 2, 512], F32, tag="ps", name="pw")
            for w in range(20):
                nc.tensor.matmul(pw[:, 0, 0:256], wtile[:, 0:128], wtile[:],
                                 start=True, stop=True)

            pro = {}

            def prologue(b):
                xn8 = wk.tile([P, 4, S], F8, tag="xn8", name="xn8")
                xnr8 = wk.tile([P, 4, S], F8, tag="xnr8", name="xnr8")
                nc.sync.dma_start(out=xn8[:], in_=xn8_d[b])
                nc.sync.dma_start(out=xnr8[:], in_=xnr8_d[b])
                pro[b] = (xn8, xnr8)

            prologue(0)

            # ---- constants (merged single-descriptor loads, by first use)
            # xn8(0)/xnr8(0) are emitted first (prologue above) so the first
            # uv matmuls aren't queued behind 2.2MB of weights on the serial
            # DMA fabric; weights stream in base -> u -> v chunks so compute
            # starts as soon as each chunk lands.
            uvb8 = cst.tile([P, 4, SD], F8, tag="uvb8", name="uvb8")
            uvbr8 = cst.tile([P, 4, SD], F8, tag="uvbr8", name="uvbr8")
            uvu8 = cst.tile([P, 4, E], F8, tag="uvu8", name="uvu8")
            uvur8 = cst.tile([P, 4, E], F8, tag="uvur8", name="uvur8")
            uvv8 = cst.tile([P, 4, E], F8, tag="uvv8", name="uvv8")
            uvvr8 = cst.tile([P, 4, E], F8, tag="uvvr8", name="uvvr8")
            ropeT = cst.tile([P, 6, S], BF16, tag="ropeT", name="ropeT")
            biasT = cst.tile([P, 4, S], BF16, tag="biasT", name="biasT")
            identr = cst.tile([P, P], BF16, tag="identr", name="identr")
            ow8 = cst.tile([P, 8, H], F8, tag="ow8", name="ow8")
            owr8 = cst.tile([P, 8, H], F8, tag="owr8", name="owr8")
            nc.sync.dma_start(out=uvb8[:], in_=uvb8_d[:])
            nc.scalar.dma_start(out=uvbr8[:], in_=uvbr8_d[:])
            nc.scalar.dma_start(out=identr[:], in_=idr_d[:])
            nc.sync.dma_start(out=uvu8[:], in_=uvu8_d[:])
            nc.scalar.dma_start(out=uvur8[:], in_=uvur8_d[:])
            nc.sync.dma_start(out=uvv8[:], in_=uvv8_d[:])
            nc.scalar.dma_start(out=uvvr8[:], in_=uvvr8_d[:])
            nc.scalar.dma_start(out=ropeT[:], in_=rope_d[:])
            nc.scalar.dma_start(out=biasT[:], in_=biasT_d[:])
            nc.scalar.dma_start(out=ow8[:], in_=ow8_d[:])
            nc.scalar.dma_start(out=owr8[:], in_=owr8_d[:])
            ccq, ssq = ropeT[:, 0, :], ropeT[:, 1, :]
            cck, ssk = ropeT[:, 2, :], ropeT[:, 3, :]
            bq, bk = ropeT[:, 4, :], ropeT[:, 5, :]

            st_a = {}
            st_k = {}
            st_g = {}

            def phase_a(b):
                """projection phase: base/rope first, then u, then v"""
                xn8, xnr8 = pro.pop(b)

                def uv_seq(whi, wlo):
                    return ([(whi, xn8, k) for k in range(2)]
                            + [(wlo, xn8, k) for k in range(2)]
                            + [(whi, xnr8, k) for k in range(2)])

                # base projection (cols 2E:2E+P) + gamma-folded rope
                psb = pp.tile([P, 2, 512], F32, tag="ps", name="ps")
                for i, (w, a, k) in enumerate(uv_seq(uvb8, uvbr8)):
                    nc.tensor.matmul(
                        psb[:, 0, :], w[:, 2 * k:2 * k + 2, :],
                        a[:, 2 * k:2 * k + 2, :],
                        start=(i == 0), stop=(i == 5), perf_mode=DR)
                baseT = wk.tile([P, S], BF16, tag="baseT", name="baseT")
                nc.scalar.activation(out=baseT[:], in_=psb[:, 0, :],
                                     func=AF.Silu, scale=1 / 16.)
                base_sw = wk.tile([P, S], BF16, tag="base_sw", name="base_sw")
                nc.sync.dma_start(out=base_sw[0:64, :], in_=baseT[64:128, :])
                nc.sync.dma_start(out=base_sw[64:128, :], in_=baseT[0:64, :])
                qT = wk.tile([P, S], BF16, tag="qT", name="qT")
                kTt = wk.tile([P, S], BF16, tag="kT", name="kT")
                for hd, (Ct, St_, Bt, dst) in enumerate(
                        ((ccq, ssq, bq, qT), (cck, ssk, bk, kTt))):
                    m1 = wk.tile([P, S], BF16, tag="m1", name="m1")
                    m2 = wk.tile([P, S], BF16, tag="m2", name="m2")
                    nc.vector.tensor_tensor(out=m1[:], in0=baseT[:], in1=Ct,
                                            op=ALU.mult)
                    nc.vector.tensor_tensor(out=m2[:], in0=base_sw[:], in1=St_,
                                            op=ALU.mult)
                    nc.vector.tensor_tensor(out=m1[:], in0=m1[:], in1=m2[:],
                                            op=ALU.add)
                    nc.vector.tensor_tensor(out=dst[:], in0=m1[:], in1=Bt,
                                            op=ALU.add)

                uT = [wk.tile([P, 2, S], BF16, tag=f"uT{fp}", name=f"uT{fp}") for fp in range(4)]
                for fp in range(4):
                    ps = pp.tile([P, 2, 512], F32, tag="ps", name="ps")
                    for hf in range(2):
                        f = 2 * fp + hf
                        for i, (w, a, k) in enumerate(uv_seq(uvu8, uvur8)):
                            nc.tensor.matmul(
                                ps[:, hf, :], w[:, 2 * k:2 * k + 2, f * P:(f + 1) * P],
                                a[:, 2 * k:2 * k + 2, :],
                                start=(i == 0), stop=(i == 5), perf_mode=DR)
                    nc.scalar.activation(out=uT[fp][:], in_=ps[:],
                                         func=AF.Silu, scale=1 / 16.)

                # v projection, natural [s_j, e]; fp8 hi/lo
                v8 = [wk.tile([P, 2, E], F8, tag=f"v8{j2}", name=f"v8{j2}") for j2 in range(2)]
                vr8 = [wk.tile([P, 2, E], F8, tag=f"vr8{j2}", name=f"vr8{j2}") for j2 in range(2)]
                for j in range(4):
                    ps = pp.tile([P, 2, 512], F32, tag="ps", name="ps")
                    for ec in range(2):
                        seq = ([(xn8, uvv8, k) for k in range(2)]
                               + [(xn8, uvvr8, k) for k in range(2)]
                               + [(xnr8, uvv8, k) for k in range(2)])
                        for i, (a, w, k) in enumerate(seq):
                            nc.tensor.matmul(
                                ps[:, ec, :], a[:, 2 * k:2 * k + 2, j * P:(j + 1) * P],
                                w[:, 2 * k:2 * k + 2, ec * 512:(ec + 1) * 512],
                                start=(i == 0), stop=(i == 5), perf_mode=DR)
                    vf = wk.tile([P, E], BF16, tag="vf", name="vf")
                    nc.scalar.activation(out=vf[:], in_=ps[:],
                                         func=AF.Silu, scale=1 / 16.)
                    nc.vector.tensor_copy(out=v8[j // 2][:, j % 2, :], in_=vf[:])
                    nc.vector.tensor_tensor(
                        out=vr8[j // 2][:, j % 2, :], in0=vf[:],
                        in1=v8[j // 2][:, j % 2, :], op=ALU.subtract)

                if b + 1 < BPC:
                    prologue(b + 1)
                st_a[b] = (uT, v8, vr8, qT, kTt)

            st_q = {}

            def phase_qk_mm(b):
                """scoresT + bias matmuls into 1-bank tiles + per-tile relu.
                The relu lands in the Act idle window right after the silus,
                so the psq rotation (bufs=2) recycles within the batch."""
                uT, v8, vr8, qT, kTt = st_a.pop(b)
                rl = wk.tile([P, 4, S], BF16, tag="rl", name="rl")
                for j in range(4):
                    ps = pq.tile([P, 512], F32, tag="psq", name="psq")
                    nc.tensor.matmul(ps[:], identr[:], biasT[:, j, :],
                                     start=True, stop=False)
                    nc.tensor.matmul(ps[:], kTt[:, j * P:(j + 1) * P], qT[:],
                                     start=False, stop=True)
                    nc.scalar.activation(out=rl[:, j, :], in_=ps[:],
                                         func=AF.Relu, scale=1 / 32.)
                st_q[b] = (uT, v8, vr8, rl)

            def phase_qk_post(b):
                """square -> fp8 pair (DVE/Pool half, runs late)"""
                uT, v8, vr8, rl = st_q.pop(b)
                ker8 = wk.tile([P, 4, S], F8, tag="ker8", name="ker8")
                kerr8 = wk.tile([P, 4, S], F8, tag="kerr8", name="kerr8")
                kf = wk.tile([P, 4, S], BF16, tag="kf", name="kf")
                nc.vector.tensor_tensor(out=kf[:], in0=rl[:], in1=rl[:],
                                        op=ALU.mult)
                nc.vector.tensor_copy(out=ker8[:], in_=kf[:])
                nc.gpsimd.tensor_tensor(out=kerr8[:], in0=kf[:],
                                        in1=ker8[:], op=ALU.subtract)
                st_k[b] = (uT, v8, vr8, ker8, kerr8)

            def phase_attn(b):
                """attention + gating for batch b (pairs produced in phase_a(b))"""
                uT, v8, vr8, ker8, kerr8 = st_k.pop(b)
                xpair = [wk.tile([P, 2, H], F32R, tag=f"xp{sp}", name=f"xp{sp}")
                         for sp in range(2)]
                for sp in range(2):
                    nc.sync.dma_start(out=xpair[sp][:], in_=x2_d[b, sp])
                g8 = [wk.tile([P, 2, S], F8, tag=f"g8{ep}", name=f"g8{ep}") for ep in range(4)]
                gr8 = [wk.tile([P, 2, S], F8, tag=f"gr8{ep}", name=f"gr8{ep}") for ep in range(4)]
                for ep in range(4):
                    ps = pp.tile([P, 2, 512], F32, tag="ps", name="ps")
                    for hf in range(2):
                        e = 2 * ep + hf
                        seq = ([(v8[j2], ker8) for j2 in range(2)]
                               + [(vr8[j2], ker8) for j2 in range(2)]
                               + [(v8[j2], kerr8) for j2 in range(2)])
                        for i, (v_, k_) in enumerate(seq):
                            j2 = i % 2
                            nc.tensor.matmul(
                                ps[:, hf, :], v_[:, :, e * P:(e + 1) * P],
                                k_[:, 2 * j2:2 * j2 + 2, :],
                                start=(i == 0), stop=(i == len(seq) - 1),
                                perf_mode=DR)
                    gf = wk.tile([P, 2, S], BF16, tag="gf", name="gf")
                    nc.vector.tensor_tensor(out=gf[:], in0=ps[:],
                                            in1=uT[ep][:], op=ALU.mult)
                    nc.scalar.copy(out=g8[ep][:], in_=gf[:])
                    nc.gpsimd.tensor_tensor(out=gr8[ep][:], in0=gf[:],
                                            in1=g8[ep][:], op=ALU.subtract)
                st_g[b] = (g8, gr8, xpair)

            def phase_out(b):
                """output projection + residual for batch b"""
                g8, gr8, xpair = st_g.pop(b)
                for sp in range(2):
                    ps = pp.tile([P, 2, 512], F32, tag="ps", name="ps")
                    for hf in range(2):
                        st = 2 * sp + hf
                        seq = ([(g8[e2], ow8, e2) for e2 in range(4)]
                               + [(gr8[e2], ow8, e2) for e2 in range(4)]
                               + [(g8[e2], owr8, e2) for e2 in range(4)])
                        for i, (g_, o_, e2) in enumerate(seq):
                            nc.tensor.matmul(
                                ps[:, hf, :], g_[:, :, st * P:(st + 1) * P],
                                o_[:, 2 * e2:2 * e2 + 2, :],
                                start=(i == 0), stop=(i == len(seq) - 1),
                                perf_mode=DR)
                    ysb = wk.tile([P, 2, H], F32, tag=f"ysb{sp}", name=f"ysb{sp}")
                    nc.vector.tensor_tensor(out=ysb[:], in0=ps[:],
                                            in1=xpair[sp][:].bitcast(F32),
                                            op=ALU.add)
                    q = nc.sync if sp == 0 else nc.scalar
                    q.dma_start(out=y_d[b, sp], in_=ysb[:])

            for b in range(BPC + 2):
                if b < BPC:
                    phase_a(b)
                if b < BPC:
                    phase_qk_mm(b)
                if 1 <= b <= BPC:
                    phase_attn(b - 1)
                if b >= 2:
                    phase_out(b - 2)
                if b < BPC:
                    phase_qk_post(b)

    if split:
        _split_waits(nc)
    return nc


_CACHE = {}


def _get_program(sim_compat=False):
    key = sim_compat
    if key not in _CACHE:
        _CACHE[key] = _build_program(sim_compat)
    return _CACHE[key]


def _pair8(a):
    hi = np.asarray(a, np.float32).astype(E4NP)
    lo = (a - hi.astype(np.float32)).astype(E4NP)
    return hi, lo


def _rope_tables():
    """sin/cos must replicate the reference bit-for-bit: the reference
    computes inv_freq = 10000**(arange/half) and sinus IN JAX f32, and at
    args ~5e6 a 1-ulp difference in inv_freq flips sin by O(0.5).  So the
    whole table chain runs on jax CPU, matching the reference exactly."""
    half = SD // 2
    import jax
    cpu = jax.local_devices(backend="cpu")[0]
    with jax.default_device(cpu):
        import jax.numpy as jnp
        inv_freq = 10000.0 ** (jnp.arange(half, dtype=jnp.float32) / half)
        pos = jnp.arange(S, dtype=jnp.float32)
        sinus = pos[:, None] * inv_freq[None, :]   # [S, 64]
        sin_t = np.asarray(jnp.sin(sinus)).T
        cos_t = np.asarray(jnp.cos(sinus)).T
    cc = np.concatenate([cos_t, cos_t], axis=0)      # [128, S]
    ss2 = np.concatenate([-sin_t, sin_t], axis=0)    # [-sin; +sin]
    return cc.astype(np.float32), ss2.astype(np.float32)


def _host_prep(ln_g, uv_w, uv_b, gamma, beta, w_bias, o_w, o_b):
    assert np.all(uv_b == 0.0), "kernel assumes uv_b == 0"
    assert np.all(o_b == 0.0), "kernel assumes o_b == 0"
    uvwTs = (uv_w.astype(np.float64) * float(ln_g[0]) * 16.0).T.astype(np.float32)  # [H, UV]
    uvr = uvwTs.reshape(2, 2, P, UV).transpose(2, 0, 1, 3)  # [p, k, slot, c]
    uvw8, uvwr8 = _pair8(uvr)
    ows = (o_w.astype(np.float64) * 8.0).T.astype(np.float32)  # [E, H]
    owr = ows.reshape(4, 2, P, H).transpose(2, 0, 1, 3)        # [p, e2, slot, h]
    ow8, owr8 = _pair8(owr)
    jj = np.arange(S)[:, None]
    ii = np.arange(S)[None, :]
    biasT = w_bias[jj - ii + S - 1].astype(np.float32)         # [t, s]
    biasT = biasT.reshape(2, 2, P, S).transpose(2, 0, 1, 3)    # [p, j2, jh, s]
    cc, ss2 = _rope_tables()

    def _sw(v):
        return np.concatenate([v[SD // 2:], v[:SD // 2]])
    bq = beta[0][:, None] * cc + _sw(beta[0])[:, None] * ss2
    bk = beta[1][:, None] * cc + _sw(beta[1])[:, None] * ss2
    ropeT = np.stack([gamma[0][:, None] * cc, _sw(gamma[0])[:, None] * ss2,
                      gamma[1][:, None] * cc, _sw(gamma[1])[:, None] * ss2,
                      bq, bk], axis=1)                          # [P, 6, S]
    uvw8 = uvw8.reshape(P, 4, UV)
    uvwr8 = uvwr8.reshape(P, 4, UV)
    return {
        "uvb8": np.ascontiguousarray(uvw8[:, :, 2 * E:]),
        "uvbr8": np.ascontiguousarray(uvwr8[:, :, 2 * E:]),
        "uvu8": np.ascontiguousarray(uvw8[:, :, :E]),
        "uvur8": np.ascontiguousarray(uvwr8[:, :, :E]),
        "uvv8": np.ascontiguousarray(uvw8[:, :, E:2 * E]),
        "uvvr8": np.ascontiguousarray(uvwr8[:, :, E:2 * E]),
        "ow8": np.ascontiguousarray(ow8.reshape(P, 8, H)),
        "owr8": np.ascontiguousarray(owr8.reshape(P, 8, H)),
        "biasT": np.ascontiguousarray(biasT.reshape(P, 4, S)).astype(BF16NP),
        "identr": np.eye(P, dtype=np.float32).astype(BF16NP),
        "ropeT": np.ascontiguousarray(ropeT).astype(BF16NP),
    }


def kernel(x, ln_g, uv_w, uv_b, gamma, beta, w_bias, o_w, o_b):
    x = np.asarray(x, dtype=np.float32)
    consts = _host_prep(np.asarray(ln_g), np.asarray(uv_w), np.asarray(uv_b),
                        np.asarray(gamma), np.asarray(beta),
                        np.asarray(w_bias), np.asarray(o_w), np.asarray(o_b))
    nc = _get_program(sim_compat=False)
    nrm = np.sqrt(np.einsum("bsh,bsh->bs", x, x, dtype=np.float32,
                            optimize=True)) * np.float32(H ** -0.5)
    inv = (1.0 / np.maximum(nrm, np.float32(EPS))).astype(np.float32)
    xn = x * inv[:, :, None]
    xnT = np.ascontiguousarray(xn.transpose(0, 2, 1))  # [B, H, S] f32
    xnr = xnT.reshape(B, 2, 2, P, S).transpose(0, 3, 1, 2, 4)  # [B, p, k, slot, s]
    xn8, xnr8 = _pair8(xnr)
    xn8 = np.ascontiguousarray(xn8.reshape(B, P, 4, S))
    xnr8 = np.ascontiguousarray(xnr8.reshape(B, P, 4, S))
    x2 = np.ascontiguousarray(
        x.reshape(B, 2, 2, P, H).transpose(0, 1, 3, 2, 4))  # [B, sp, p, hf, h]
    in_maps = []
    for c in range(N_CORES):
        m = dict(consts)
        m["x2"] = np.ascontiguousarray(x2[c * BPC:(c + 1) * BPC])
        m["xn8"] = xn8[c * BPC:(c + 1) * BPC]
        m["xnr8"] = xnr8[c * BPC:(c + 1) * BPC]
        in_maps.append(m)
    res = run_bass_kernel_spmd(nc, in_maps, core_ids=list(range(N_CORES)))
    y = np.concatenate([r["y8"] for r in res.results], axis=0)  # [B,2,P,2,H]
    out = y.reshape(B, 2, P, 2, H).transpose(0, 1, 3, 2, 4).reshape(B, S, H)
    return np.ascontiguousarray(out.astype(np.float32))


# revision 62
# speedup vs baseline: 1.1027x; 1.0937x over previous
"""GAU-alpha (gated attention unit) Trainium2 kernel, fp8 DoubleRow edition.

Data-parallel over batch: 64 batches -> 8 NeuronCores x 8 batches.
Per-batch pipeline on-chip:
  (host) ScaleNorm + transpose + fp8 hi/lo split of xn ->
  fused uv projection + SiLU -> gamma-folded RoPE (DVE) ->
  relu^2 relative-position attention -> gating -> output projection + residual.

All large matmuls run as fp8e4m3 DoubleRow instructions (2 (weight, ifmap)
slot-pairs per instruction).  Each operand X is split into hi+lo fp8 tiles
(X ~= X8 + Xr8, bf16-level accuracy); each K=128 block contributes 3 pairs:
(W8,A8), (Wr8,A8), (W8,Ar8).  The qk matmul stays bf16 (small).  Scales:
uvw*16 (silu scale 1/16), relu scale 1/32 -> ker_dev = ker_ref/8, o_w*8,
so no epilogue descale is needed.

Engine layout per batch (cost-model):  PE ~22.8us (uv 10.9 + qk 1.7 +
attn 5.1 + out 5.1), DVE ~19us (rope TTs, relu^2 square, fp8 copies +
v/ker residual subs, gating mult), Act ~15us (SiLUs, relu, g8 copies),
Pool ~13us (g residual subs, residual adds).  Phase order per iteration
is a(b), attn(b-1), out(b-2), qk(b): qk last gives its Act/DVE epilogue
a full iteration of slack before attn(b) consumes ker8(b), and gf(b-1)
lands early enough on DVE to recycle PSUM banks for out(b-2).
DMAs are merged via host-side layouts (one descriptor per logical load)
because each HWDGE trigger costs ~0.6-1.3us of serial queue time.
"""
import numpy as np
import ml_dtypes

import concourse.bass as bass
import concourse.tile as tile
from concourse import mybir
from concourse.bass_utils import run_bass_kernel_spmd

F32 = mybir.dt.float32
F32R = mybir.dt.float32r
BF16 = mybir.dt.bfloat16
F8 = mybir.dt.float8e4
E4NP = ml_dtypes.float8_e4m3
BF16NP = ml_dtypes.bfloat16

B, S, H = 64, 512, 512
E = 1024          # expansion dim
SD = 128          # attention head width s
UV = 2 * E + SD   # 2176
N_CORES = 8
BPC = B // N_CORES
P = 128
EPS = 1e-5
DR = mybir.MatmulPerfMode.DoubleRow


def _split_waits(nc, max_waits=1):
    """This walrus build rejects >1 sync-wait on CTRL-encoded instructions
    (Drain/NoOp); Tile's exit drain always violates that.  Split any
    instruction carrying more than `max_waits` waits into a chain of
    single-wait NoOps on the same engine."""
    ctr = 0
    for f in nc.m.functions:
        for bb in f.blocks:
            new_insts = []
            for ins in bb.instructions:
                si = ins.sync_info
                if si is not None and si.on_wait and len(si.on_wait) > max_waits:
                    waits = list(si.on_wait)
                    head, tail = waits[:-max_waits], waits[-max_waits:]
                    for w in head:
                        ctr += 1
                        nop = mybir.InstNoOp(
                            name=f"I-waitsplit-{ctr}",
                            ins=[], outs=[],
                            sync_info=mybir.SyncInfo(on_wait=[w], on_update=[]),
                        )
                        nop.engine = ins.engine
                        new_insts.append(nop)
                    si.on_wait = tail
                new_insts.append(ins)
            bb.instructions = new_insts
    return ctr


def _build_program(sim_compat=False, split=True):
    nc = bass.Bass()
    AF = mybir.ActivationFunctionType
    ALU = mybir.AluOpType

    x2_d = nc.dram_tensor("x2", [BPC, 2, P, 2, H], F32R, kind="ExternalInput")
    xn8_d = nc.dram_tensor("xn8", [BPC, P, 4, S], F8, kind="ExternalInput")
    xnr8_d = nc.dram_tensor("xnr8", [BPC, P, 4, S], F8, kind="ExternalInput")
    uvb8_d = nc.dram_tensor("uvb8", [P, 4, SD], F8, kind="ExternalInput")
    uvbr8_d = nc.dram_tensor("uvbr8", [P, 4, SD], F8, kind="ExternalInput")
    uvu8_d = nc.dram_tensor("uvu8", [P, 4, E], F8, kind="ExternalInput")
    uvur8_d = nc.dram_tensor("uvur8", [P, 4, E], F8, kind="ExternalInput")
    uvv8_d = nc.dram_tensor("uvv8", [P, 4, E], F8, kind="ExternalInput")
    uvvr8_d = nc.dram_tensor("uvvr8", [P, 4, E], F8, kind="ExternalInput")
    ow8_d = nc.dram_tensor("ow8", [P, 8, H], F8, kind="ExternalInput")
    owr8_d = nc.dram_tensor("owr8", [P, 8, H], F8, kind="ExternalInput")
    biasT_d = nc.dram_tensor("biasT", [P, 4, S], F8, kind="ExternalInput")
    rope_d = nc.dram_tensor("ropeT", [P, 6, S], BF16, kind="ExternalInput")
    idr_d = nc.dram_tensor("identr", [P, P], F8, kind="ExternalInput")
    idrf_d = nc.dram_tensor("identrf", [P, P], F32R, kind="ExternalInput")
    y_d = nc.dram_tensor("y8", [BPC, 2, P, 2, H], F32, kind="ExternalOutput")

    with tile.TileContext(nc) as tc:
        with (
            tc.tile_pool(name="const", bufs=1) as cst,
            tc.tile_pool(name="work", bufs=2) as wk,
            tc.tile_pool(name="ps", bufs=3, space="PSUM") as pp,
            tc.tile_pool(name="psq", bufs=2, space="PSUM") as pq,
        ):
            # ---- PE warm-up: keep the tensor engine streaming while the
            # first DMAs land so the p-state ramp is done before real work.
            # wtile is memset on-chip (no DMA) so warm-up starts ~immediately.
            wtile = cst.tile([P, 256], BF16, tag="wtile", name="wtile")
            nc.gpsimd.memset(wtile[:], 0)
            pw = pp.tile([P, 2, 512], F32, tag="ps", name="pw")
            for w in range(20):
                nc.tensor.matmul(pw[:, 0, 0:256], wtile[:, 0:128], wtile[:],
                                 start=True, stop=True)

            pro = {}

            def prologue(b):
                xn8 = wk.tile([P, 4, S], F8, tag="xn8", name="xn8")
                xnr8 = wk.tile([P, 4, S], F8, tag="xnr8", name="xnr8")
                nc.sync.dma_start(out=xn8[:], in_=xn8_d[b])
                nc.sync.dma_start(out=xnr8[:], in_=xnr8_d[b])
                pro[b] = (xn8, xnr8)

            prologue(0)

            # ---- constants (merged single-descriptor loads, by first use)
            # xn8(0)/xnr8(0) are emitted first (prologue above) so the first
            # uv matmuls aren't queued behind 2.2MB of weights on the serial
            # DMA fabric; weights stream in base -> u -> v chunks so compute
            # starts as soon as each chunk lands.
            uvb8 = cst.tile([P, 4, SD], F8, tag="uvb8", name="uvb8")
            uvbr8 = cst.tile([P, 4, SD], F8, tag="uvbr8", name="uvbr8")
            uvu8 = cst.tile([P, 4, E], F8, tag="uvu8", name="uvu8")
            uvur8 = cst.tile([P, 4, E], F8, tag="uvur8", name="uvur8")
            uvv8 = cst.tile([P, 4, E], F8, tag="uvv8", name="uvv8")
            uvvr8 = cst.tile([P, 4, E], F8, tag="uvvr8", name="uvvr8")
            ropeT = cst.tile([P, 6, S], BF16, tag="ropeT", name="ropeT")
            biasT = cst.tile([P, 4, S], F8, tag="biasT", name="biasT")
            identr = cst.tile([P, P], F8, tag="identr", name="identr")
            identrf = cst.tile([P, P], F32R, tag="identrf", name="identrf")
            ow8 = cst.tile([P, 8, H], F8, tag="ow8", name="ow8")
            owr8 = cst.tile([P, 8, H], F8, tag="owr8", name="owr8")
            nc.sync.dma_start(out=uvb8[:], in_=uvb8_d[:])
            nc.scalar.dma_start(out=uvbr8[:], in_=uvbr8_d[:])
            nc.sync.dma_start(out=uvu8[:], in_=uvu8_d[:])
            nc.scalar.dma_start(out=uvur8[:], in_=uvur8_d[:])
            nc.sync.dma_start(out=uvv8[:], in_=uvv8_d[:])
            nc.scalar.dma_start(out=uvvr8[:], in_=uvvr8_d[:])
            nc.scalar.dma_start(out=biasT[:], in_=biasT_d[:])
            nc.scalar.dma_start(out=identr[:], in_=idr_d[:])
            nc.scalar.dma_start(out=ropeT[:], in_=rope_d[:])
            nc.scalar.dma_start(out=identrf[:], in_=idrf_d[:])
            nc.scalar.dma_start(out=ow8[:], in_=ow8_d[:])
            nc.scalar.dma_start(out=owr8[:], in_=owr8_d[:])
            ccq, ssq = ropeT[:, 0, :], ropeT[:, 1, :]
            cck, ssk = ropeT[:, 2, :], ropeT[:, 3, :]
            bq, bk = ropeT[:, 4, :], ropeT[:, 5, :]

            st_a = {}
            st_k = {}
            st_g = {}

            def phase_a(b):
                """projection phase: base/rope first, then u, then v"""
                xn8, xnr8 = pro.pop(b)

                def uv_seq(whi, wlo):
                    return ([(whi, xn8, k) for k in range(2)]
                            + [(wlo, xn8, k) for k in range(2)]
                            + [(whi, xnr8, k) for k in range(2)])

                # base projection (cols 2E:2E+P) + gamma-folded rope
                psb = pp.tile([P, 2, 512], F32, tag="ps", name="ps")
                for i, (w, a, k) in enumerate(uv_seq(uvb8, uvbr8)):
                    nc.tensor.matmul(
                        psb[:, 0, :], w[:, 2 * k:2 * k + 2, :],
                        a[:, 2 * k:2 * k + 2, :],
                        start=(i == 0), stop=(i == 5), perf_mode=DR)
                baseT = wk.tile([P, S], BF16, tag="baseT", name="baseT")
                nc.scalar.activation(out=baseT[:], in_=psb[:, 0, :],
                                     func=AF.Silu, scale=1 / 16.)
                base_sw = wk.tile([P, S], BF16, tag="base_sw", name="base_sw")
                nc.sync.dma_start(out=base_sw[0:64, :], in_=baseT[64:128, :])
                nc.sync.dma_start(out=base_sw[64:128, :], in_=baseT[0:64, :])
                qT = wk.tile([P, S], BF16, tag="qT", name="qT")
                kTt = wk.tile([P, S], BF16, tag="kT", name="kT")
                for hd, (Ct, St_, Bt, dst) in enumerate(
                        ((ccq, ssq, bq, qT), (cck, ssk, bk, kTt))):
                    m1 = wk.tile([P, S], BF16, tag="m1", name="m1")
                    m2 = wk.tile([P, S], BF16, tag="m2", name="m2")
                    nc.vector.tensor_tensor(out=m1[:], in0=baseT[:], in1=Ct,
                                            op=ALU.mult)
                    nc.vector.tensor_tensor(out=m2[:], in0=base_sw[:], in1=St_,
                                            op=ALU.mult)
                    nc.vector.tensor_tensor(out=m1[:], in0=m1[:], in1=m2[:],
                                            op=ALU.add)
                    nc.vector.tensor_tensor(out=dst[:], in0=m1[:], in1=Bt,
                                            op=ALU.add)

                uT = [wk.tile([P, 2, S], BF16, tag=f"uT{fp}", name=f"uT{fp}") for fp in range(4)]
                for fp in range(4):
                    ps = pp.tile([P, 2, 512], F32, tag="ps", name="ps")
                    for hf in range(2):
                        f = 2 * fp + hf
                        for i, (w, a, k) in enumerate(uv_seq(uvu8, uvur8)):
                            nc.tensor.matmul(
                                ps[:, hf, :], w[:, 2 * k:2 * k + 2, f * P:(f + 1) * P],
                                a[:, 2 * k:2 * k + 2, :],
                                start=(i == 0), stop=(i == 5), perf_mode=DR)
                    nc.scalar.activation(out=uT[fp][:], in_=ps[:],
                                         func=AF.Silu, scale=1 / 16.)

                # v projection, natural [s_j, e]; fp8 hi/lo
                v8 = [wk.tile([P, 2, E], F8, tag=f"v8{j2}", name=f"v8{j2}") for j2 in range(2)]
                vr8 = [wk.tile([P, 2, E], F8, tag=f"vr8{j2}", name=f"vr8{j2}") for j2 in range(2)]
                for j in range(4):
                    ps = pp.tile([P, 2, 512], F32, tag="ps", name="ps")
                    for ec in range(2):
                        seq = ([(xn8, uvv8, k) for k in range(2)]
                               + [(xn8, uvvr8, k) for k in range(2)]
                               + [(xnr8, uvv8, k) for k in range(2)])
                        for i, (a, w, k) in enumerate(seq):
                            nc.tensor.matmul(
                                ps[:, ec, :], a[:, 2 * k:2 * k + 2, j * P:(j + 1) * P],
                                w[:, 2 * k:2 * k + 2, ec * 512:(ec + 1) * 512],
                                start=(i == 0), stop=(i == 5), perf_mode=DR)
                    vf = wk.tile([P, E], BF16, tag="vf", name="vf")
                    nc.scalar.activation(out=vf[:], in_=ps[:],
                                         func=AF.Silu, scale=1 / 16.)
                    nc.vector.tensor_copy(out=v8[j // 2][:, j % 2, :], in_=vf[:])
                    nc.vector.tensor_tensor(
                        out=vr8[j // 2][:, j % 2, :], in0=vf[:],
                        in1=v8[j // 2][:, j % 2, :], op=ALU.subtract)

                if b + 1 < BPC:
                    prologue(b + 1)
                st_a[b] = (uT, v8, vr8, qT, kTt)

            st_q = {}

            def phase_qk_mm(b):
                """scoresT + bias matmuls into 1-bank tiles + per-tile relu.
                The relu lands in the Act idle window right after the silus,
                so the psq rotation (bufs=2) recycles within the batch."""
                uT, v8, vr8, qT, kTt = st_a.pop(b)
                rl = wk.tile([P, 4, S], BF16, tag="rl", name="rl")
                for j in range(4):
                    ps = pq.tile([P, 512], F32, tag="psq", name="psq")
                    nc.tensor.matmul(ps[:], identr[:], biasT[:, j, :],
                                     start=True, stop=False)
                    nc.tensor.matmul(ps[:], kTt[:, j * P:(j + 1) * P], qT[:],
                                     start=False, stop=True)
                    nc.scalar.activation(out=rl[:, j, :], in_=ps[:],
                                         func=AF.Relu, scale=1 / 32.)
                st_q[b] = (uT, v8, vr8, rl)

            def phase_qk_post(b):
                """square -> fp8 pair (DVE/Pool half, runs late)"""
                uT, v8, vr8, rl = st_q.pop(b)
                ker8 = wk.tile([P, 4, S], F8, tag="ker8", name="ker8")
                kerr8 = wk.tile([P, 4, S], F8, tag="kerr8", name="kerr8")
                kf = wk.tile([P, 4, S], BF16, tag="kf", name="kf")
                nc.vector.tensor_tensor(out=kf[:], in0=rl[:], in1=rl[:],
                                        op=ALU.mult)
                nc.vector.tensor_copy(out=ker8[:], in_=kf[:])
                q = nc.vector if b == BPC - 1 else nc.gpsimd
                q.tensor_tensor(out=kerr8[:], in0=kf[:],
                                in1=ker8[:], op=ALU.subtract)
                st_k[b] = (uT, v8, vr8, ker8, kerr8)

            def phase_attn(b):
                """attention + gating for batch b (pairs produced in phase_a(b))"""
                uT, v8, vr8, ker8, kerr8 = st_k.pop(b)
                xpair = [wk.tile([P, 2, H], F32R, tag=f"xp{sp}", name=f"xp{sp}")
                         for sp in range(2)]
                for sp in range(2):
                    nc.scalar.dma_start(out=xpair[sp][:], in_=x2_d[b, sp])
                g8 = [wk.tile([P, 2, S], F8, tag=f"g8{ep}", name=f"g8{ep}") for ep in range(4)]
                gr8 = [wk.tile([P, 2, S], F8, tag=f"gr8{ep}", name=f"gr8{ep}") for ep in range(4)]
                for ep in range(4):
                    ps = pp.tile([P, 2, 512], F32, tag="ps", name="ps")
                    for hf in range(2):
                        e = 2 * ep + hf
                        seq = ([(v8[j2], ker8) for j2 in range(2)]
                               + [(vr8[j2], ker8) for j2 in range(2)]
                               + [(v8[j2], kerr8) for j2 in range(2)])
                        for i, (v_, k_) in enumerate(seq):
                            j2 = i % 2
                            nc.tensor.matmul(
                                ps[:, hf, :], v_[:, :, e * P:(e + 1) * P],
                                k_[:, 2 * j2:2 * j2 + 2, :],
                                start=(i == 0), stop=(i == len(seq) - 1),
                                perf_mode=DR)
                    gf = wk.tile([P, 2, S], BF16, tag="gf", name="gf")
                    nc.vector.tensor_tensor(out=gf[:], in0=ps[:],
                                            in1=uT[ep][:], op=ALU.mult)
                    nc.scalar.copy(out=g8[ep][:], in_=gf[:])
                    nc.gpsimd.tensor_tensor(out=gr8[ep][:], in0=gf[:],
                                            in1=g8[ep][:], op=ALU.subtract)
                st_g[b] = (g8, gr8, xpair)

            def phase_out(b):
                """output projection + residual for batch b"""
                g8, gr8, xpair = st_g.pop(b)
                for sp in range(2):
                    ps = pp.tile([P, 2, 512], F32, tag="ps", name="ps")
                    for hf in range(2):
                        st = 2 * sp + hf
                        seq = ([(g8[e2], ow8, e2) for e2 in range(4)]
                               + [(g8[e2], owr8, e2) for e2 in range(4)]
                               + [(gr8[e2], ow8, e2) for e2 in range(4)])
                        for i, (g_, o_, e2) in enumerate(seq):
                            nc.tensor.matmul(
                                ps[:, hf, :], g_[:, :, st * P:(st + 1) * P],
                                o_[:, 2 * e2:2 * e2 + 2, :],
                                start=(i == 0), stop=False, perf_mode=DR)
                        # residual add folded into the accumulation: an
                        # identity matmul streams x through at f32r rate.
                        nc.tensor.matmul(ps[:, hf, :], identrf[:],
                                         xpair[sp][:, hf, :],
                                         start=False, stop=True)
                    ysb = wk.tile([P, 2, H], F32, tag=f"ysb{sp}", name=f"ysb{sp}")
                    nc.scalar.copy(out=ysb[:], in_=ps[:])
                    q = nc.sync if sp == 0 else nc.scalar
                    q.dma_start(out=y_d[b, sp], in_=ysb[:])

            for b in range(BPC + 2):
                if b < BPC:
                    phase_a(b)
                if b < BPC:
                    phase_qk_mm(b)
                if 1 <= b <= BPC:
                    phase_attn(b - 1)
                if b >= 2:
                    phase_out(b - 2)
                if b < BPC:
                    phase_qk_post(b)

    if split:
        _split_waits(nc)
    return nc


_CACHE = {}


def _get_program(sim_compat=False):
    key = sim_compat
    if key not in _CACHE:
        _CACHE[key] = _build_program(sim_compat)
    return _CACHE[key]


def _pair8(a):
    hi = np.asarray(a, np.float32).astype(E4NP)
    lo = (a - hi.astype(np.float32)).astype(E4NP)
    return hi, lo


def _rope_tables():
    """sin/cos must replicate the reference bit-for-bit: the reference
    computes inv_freq = 10000**(arange/half) and sinus IN JAX f32, and at
    args ~5e6 a 1-ulp difference in inv_freq flips sin by O(0.5).  So the
    whole table chain runs on jax CPU, matching the reference exactly."""
    half = SD // 2
    import jax
    cpu = jax.local_devices(backend="cpu")[0]
    with jax.default_device(cpu):
        import jax.numpy as jnp
        inv_freq = 10000.0 ** (jnp.arange(half, dtype=jnp.float32) / half)
        pos = jnp.arange(S, dtype=jnp.float32)
        sinus = pos[:, None] * inv_freq[None, :]   # [S, 64]
        sin_t = np.asarray(jnp.sin(sinus)).T
        cos_t = np.asarray(jnp.cos(sinus)).T
    cc = np.concatenate([cos_t, cos_t], axis=0)      # [128, S]
    ss2 = np.concatenate([-sin_t, sin_t], axis=0)    # [-sin; +sin]
    return cc.astype(np.float32), ss2.astype(np.float32)


def _host_prep(ln_g, uv_w, uv_b, gamma, beta, w_bias, o_w, o_b):
    assert np.all(uv_b == 0.0), "kernel assumes uv_b == 0"
    assert np.all(o_b == 0.0), "kernel assumes o_b == 0"
    uvwTs = (uv_w.astype(np.float64) * float(ln_g[0]) * 16.0).T.astype(np.float32)  # [H, UV]
    uvr = uvwTs.reshape(2, 2, P, UV).transpose(2, 0, 1, 3)  # [p, k, slot, c]
    uvw8, uvwr8 = _pair8(uvr)
    ows = (o_w.astype(np.float64) * 8.0).T.astype(np.float32)  # [E, H]
    owr = ows.reshape(4, 2, P, H).transpose(2, 0, 1, 3)        # [p, e2, slot, h]
    ow8, owr8 = _pair8(owr)
    jj = np.arange(S)[:, None]
    ii = np.arange(S)[None, :]
    biasT = w_bias[jj - ii + S - 1].astype(np.float32)         # [t, s]
    biasT = biasT.reshape(2, 2, P, S).transpose(2, 0, 1, 3)    # [p, j2, jh, s]
    cc, ss2 = _rope_tables()

    def _sw(v):
        return np.concatenate([v[SD // 2:], v[:SD // 2]])
    bq = beta[0][:, None] * cc + _sw(beta[0])[:, None] * ss2
    bk = beta[1][:, None] * cc + _sw(beta[1])[:, None] * ss2
    ropeT = np.stack([gamma[0][:, None] * cc, _sw(gamma[0])[:, None] * ss2,
                      gamma[1][:, None] * cc, _sw(gamma[1])[:, None] * ss2,
                      bq, bk], axis=1)                          # [P, 6, S]
    uvw8 = uvw8.reshape(P, 4, UV)
    uvwr8 = uvwr8.reshape(P, 4, UV)
    return {
        "uvb8": np.ascontiguousarray(uvw8[:, :, 2 * E:]),
        "uvbr8": np.ascontiguousarray(uvwr8[:, :, 2 * E:]),
        "uvu8": np.ascontiguousarray(uvw8[:, :, :E]),
        "uvur8": np.ascontiguousarray(uvwr8[:, :, :E]),
        "uvv8": np.ascontiguousarray(uvw8[:, :, E:2 * E]),
        "uvvr8": np.ascontiguousarray(uvwr8[:, :, E:2 * E]),
        "ow8": np.ascontiguousarray(ow8.reshape(P, 8, H)),
        "owr8": np.ascontiguousarray(owr8.reshape(P, 8, H)),
        "biasT": np.ascontiguousarray(biasT.reshape(P, 4, S)).astype(E4NP),
        "identr": np.eye(P, dtype=np.float32).astype(E4NP),
        "identrf": np.eye(P, dtype=np.float32),
        "ropeT": np.ascontiguousarray(ropeT).astype(BF16NP),
    }


def kernel(x, ln_g, uv_w, uv_b, gamma, beta, w_bias, o_w, o_b):
    x = np.asarray(x, dtype=np.float32)
    consts = _host_prep(np.asarray(ln_g), np.asarray(uv_w), np.asarray(uv_b),
                        np.asarray(gamma), np.asarray(beta),
                        np.asarray(w_bias), np.asarray(o_w), np.asarray(o_b))
    nc = _get_program(sim_compat=False)
    nrm = np.sqrt(np.einsum("bsh,bsh->bs", x, x, dtype=np.float32,
                            optimize=True)) * np.float32(H ** -0.5)
    inv = (1.0 / np.maximum(nrm, np.float32(EPS))).astype(np.float32)
    xn = x * inv[:, :, None]
    xnT = np.ascontiguousarray(xn.transpose(0, 2, 1))  # [B, H, S] f32
    xnr = xnT.reshape(B, 2, 2, P, S).transpose(0, 3, 1, 2, 4)  # [B, p, k, slot, s]
    xn8, xnr8 = _pair8(xnr)
    xn8 = np.ascontiguousarray(xn8.reshape(B, P, 4, S))
    xnr8 = np.ascontiguousarray(xnr8.reshape(B, P, 4, S))
    x2 = np.ascontiguousarray(
        x.reshape(B, 2, 2, P, H).transpose(0, 1, 3, 2, 4))  # [B, sp, p, hf, h]
    in_maps = []
    for c in range(N_CORES):
        m = dict(consts)
        m["x2"] = np.ascontiguousarray(x2[c * BPC:(c + 1) * BPC])
        m["xn8"] = xn8[c * BPC:(c + 1) * BPC]
        m["xnr8"] = xnr8[c * BPC:(c + 1) * BPC]
        in_maps.append(m)
    res = run_bass_kernel_spmd(nc, in_maps, core_ids=list(range(N_CORES)))
    y = np.concatenate([r["y8"] for r in res.results], axis=0)  # [B,2,P,2,H]
    out = y.reshape(B, 2, P, 2, H).transpose(0, 1, 3, 2, 4).reshape(B, S, H)
    return np.ascontiguousarray(out.astype(np.float32))
